# revision 17
# baseline (speedup 1.0000x reference)
"""Multi-head attention (B=2, S=2048, D=2048, H=16) on 8 TRN2 NeuronCores.

Tensor-parallel over heads: each core computes 2 of 16 heads end to end
(q/k/v projections column-sharded, out_proj row-sharded) and writes a
partial output; the host sums the 8 partials and adds the output bias.

Per-core device pipeline (b = batch index, looped):
  phase 1: q^T,k^T ([HD, S] layout) and v ([S, HD] layout) via matmuls
           against host-pre-transposed hidden^T; biases are added with a
           K=1 matmul against a ones row. k is PE-transposed to [S, HD]
           for the k output.
  phase 2 (fused with out_proj, per 512-wide s_q strip):
           scores^T[s_k, s_q] = k^T.T @ q^T (1/sqrt(HD) folded into Wq),
           + mask^T on diagonal blocks (DVE), exp (ACT, bf16 out), then
           ctx^T[d, s_q] += V_chunk.T @ exp(S^T) and a denominator row
           denom[1, s_q] += ones.T @ exp(S^T), both accumulated in PSUM
           chunk by chunk. 1/denom is broadcast across partitions with a
           K=1 matmul and ctx^T is normalized by one DVE multiply; the
           out_proj partial for the strip follows immediately.

Matmuls for projections/scores/out_proj run in float32r (single-pass
fp32 on the PE at bf16 rate); the PV matmul runs in bf16 on softmax
weights in [0,1]. All accumulation is fp32 in PSUM. A causal-specialized
program skips fully-masked score blocks; any other mask takes a general
program with identical math.
"""

import os
import sys
import types

for _p in ("/opt/trn_rl_repo", "/root/.axon_site/_ro/trn_rl_repo", "/root/.axon_site"):
    if os.path.isdir(_p) and _p not in sys.path:
        sys.path.append(_p)

import numpy as np

import concourse.bass as bass
import concourse.mybir as mybir
import concourse.tile as tile
from concourse.bass_utils import run_bass_kernel_spmd

# problem shape (hardcoded per contest contract)
B, S, D, H, HD = 2, 2048, 2048, 16, 128
NCORES = 8
HPC = H // NCORES          # heads per core = 2
DSH = HPC * HD             # per-core model-dim shard = 256
P = 128                    # partitions
NCH = D // P               # contraction chunks = 16
NSB = S // P               # 128-row blocks per batch = 16
SQW = 512                  # matmul free width
NSTW = S // SQW            # strips per batch = 4
BLK = SQW // P             # 128-blocks per strip = 4
CG = 4                     # hidden chunks per packed DMA

F32 = mybir.dt.float32
F32R = mybir.dt.float32r   # projection/scores/out_proj matmul dtype
PV_DT = mybir.dt.bfloat16  # exp(S) and V dtypes for the PV matmul
AF = mybir.ActivationFunctionType

MASK_CLAMP = -60.0


def _split_multiwait_insts(nc):
    """This walrus build rejects any instruction carrying more than one
    sync-wait command (seen on Drain/CTRL and Matmult/S3_LW). Tile
    occasionally aggregates several waits onto one instruction. Hoist all
    but the last wait onto preceding NoOps on the same engine."""
    n = [0]

    def fresh(base):
        n[0] += 1
        return f"{base}_wsplit{n[0]}"

    for fn in nc.m.functions:
        for blk in fn.blocks:
            out = []
            for inst in blk.instructions:
                si = getattr(inst, "sync_info", None)
                if si is not None and len(si.on_wait) > 1:
                    waits = list(si.on_wait)
                    for w in waits[:-1]:
                        d = mybir.InstNoOp(name=fresh(inst.name), ins=[], outs=[])
                        d.engine = inst.engine
                        d.sync_info = mybir.SyncInfo(on_wait=[w], on_update=[])
                        out.append(d)
                    inst.sync_info = mybir.SyncInfo(
                        on_wait=[waits[-1]], on_update=list(si.on_update)
                    )
                out.append(inst)
            blk.instructions[:] = out


def build_program(causal=True, split=True):
    nc = bass.Bass(target_bir_lowering=False, num_swdge_queues=4)

    hT = nc.dram_tensor("hT", [P, B * NSTW * NCH * SQW], F32R,
                        kind="ExternalInput")
    mT = nc.dram_tensor("mT", [S, S], F32, kind="ExternalInput")
    # diagonal mask blocks, packed [128, NSTW*BLK*SQW] (causal path)
    mdg = nc.dram_tensor("mdg", [P, NSTW * BLK * SQW], F32, kind="ExternalInput")
    # projection weights, packed [128, NCH*DSH] chunk-major
    wqT = nc.dram_tensor("wqT", [P, NCH * DSH], F32R, kind="ExternalInput")
    wkT = nc.dram_tensor("wkT", [P, NCH * DSH], F32R, kind="ExternalInput")
    wvT = nc.dram_tensor("wvT", [P, NCH * DSH], F32R, kind="ExternalInput")
    woT = nc.dram_tensor("woT", [DSH, D], F32R, kind="ExternalInput")
    # rows: bq (pre-scaled), bk, bv, ones
    bqv = nc.dram_tensor("bqv", [4, SQW], F32R, kind="ExternalInput")
    ident_d = nc.dram_tensor("ident_d", [P, P], F32, kind="ExternalInput")
    outp = nc.dram_tensor("outp", [B * S, D], F32, kind="ExternalOutput")
    k_out = nc.dram_tensor("k_out", [B, S, DSH], F32, kind="ExternalOutput")
    v_out = nc.dram_tensor("v_out", [B, S, DSH], F32, kind="ExternalOutput")

    with tile.TileContext(nc) as tc:
        from contextlib import ExitStack

        with ExitStack() as top:
            constp = top.enter_context(tc.tile_pool(name="const", bufs=1))
            wproj = top.enter_context(tc.tile_pool(name="wproj", bufs=1))
            qkvp = top.enter_context(tc.tile_pool(name="qkv", bufs=1))
            streamp = top.enter_context(tc.tile_pool(name="stream", bufs=2))
            expsp = top.enter_context(tc.tile_pool(name="exps", bufs=1))
            maskp = top.enter_context(tc.tile_pool(name="mask", bufs=3))
            stagep = top.enter_context(tc.tile_pool(name="stage", bufs=3))

            identity = constp.tile([P, P], F32, name="identity", tag="identity")
            nc.sync.dma_start(identity[:], ident_d[:, :])

            wq_t = wproj.tile([P, NCH * DSH], F32R, name="wq_t", tag="wq_t")
            wk_t = wproj.tile([P, NCH * DSH], F32R, name="wk_t", tag="wk_t")
            wv_t = wproj.tile([P, NCH * DSH], F32R, name="wv_t", tag="wv_t")
            for t, dram in ((wq_t, wqT), (wk_t, wkT), (wv_t, wvT)):
                nc.sync.dma_start(t[:], dram[:, :])

            bq_t = constp.tile([1, SQW], F32R, name="bq_t", tag="bq_t")
            bk_t = constp.tile([1, SQW], F32R, name="bk_t", tag="bk_t")
            bv_t = constp.tile([1, SQW], F32R, name="bv_t", tag="bv_t")
            ones_row = constp.tile([1, SQW], F32R, name="ones_row", tag="ones_row")
            for i, t in enumerate((bq_t, bk_t, bv_t, ones_row)):
                nc.sync.dma_start(t[:], bqv[i:i + 1, :])
            ones_col = constp.tile([P, 1], PV_DT, name="ones_col", tag="ones_col")
            nc.vector.memset(ones_col[:], 1.0)

            # out_proj weights and the resident diagonal mask load later --
            # first needed well into batch 0
            wo_t = [constp.tile([P, D], F32R, name=f"woT{h}", tag=f"woT{h}")
                    for h in range(HPC)]


            def wslice(t, c):
                return t[:, c * DSH:(c + 1) * DSH]

            for b in range(B):
                qT = [qkvp.tile([P, S], F32R, name=f"qT{h}", tag=f"qT{h}")
                      for h in range(HPC)]
                kT = [qkvp.tile([P, S], F32R, name=f"kT{h}", tag=f"kT{h}")
                      for h in range(HPC)]
                vbf = [qkvp.tile([P, NCH * HD], PV_DT,
                                 name=f"vbf{h}", tag=f"vbf{h}")
                       for h in range(HPC)]
                ctxT = [qkvp.tile([P, S], F32R, name=f"ctxT{h}", tag=f"ctxT{h}")
                        for h in range(HPC)]

                # ---- phase 1: projections ----
                with tc.tile_pool(name="ps1", bufs=1, space="PSUM") as ps1, \
                     tc.tile_pool(name="ps1t", bufs=2, space="PSUM") as ps1t:
                    for st in range(NSTW):
                        s0 = st * SQW
                        psq = [ps1.tile([P, SQW], F32, name=f"psq{h}", tag=f"psq{h}")
                               for h in range(HPC)]
                        psk = [ps1.tile([P, SQW], F32, name=f"psk{h}", tag=f"psk{h}")
                               for h in range(HPC)]
                        psv = [ps1.tile([P, SQW], F32, name=f"psv{i}", tag=f"psv{i}")
                               for i in range(2)]
                        base = (b * NSTW + st) * NCH * SQW
                        for cg in range(NCH // CG):
                            hc = streamp.tile([P, CG * SQW], F32R,
                                              name="hchunk", tag="hchunk")
                            nc.sync.dma_start(
                                hc[:], hT[:, base + cg * CG * SQW:
                                          base + (cg + 1) * CG * SQW])
                            for ci in range(CG):
                                c = cg * CG + ci
                                hcs = hc[:, ci * SQW:(ci + 1) * SQW]
                                for h in range(HPC):
                                    hsl = slice(h * HD, (h + 1) * HD)
                                    nc.tensor.matmul(
                                        psq[h][:], wslice(wq_t, c)[:, hsl], hcs,
                                        start=(c == 0), stop=False)
                                    nc.tensor.matmul(
                                        psk[h][:], wslice(wk_t, c)[:, hsl], hcs,
                                        start=(c == 0), stop=False)
                                for s2 in range(BLK):
                                    nc.tensor.matmul(
                                        psv[s2 // 2][:, (s2 % 2) * DSH:
                                                     (s2 % 2 + 1) * DSH],
                                        hcs[:, s2 * P:(s2 + 1) * P],
                                        wslice(wv_t, c),
                                        start=(c == 0 and s2 % 2 == 0),
                                        stop=False)
                        for h in range(HPC):
                            hsl = slice(h * HD, (h + 1) * HD)
                            nc.tensor.matmul(psq[h][:], bq_t[:, hsl], ones_row[:],
                                             start=False, stop=True)
                            nc.tensor.matmul(psk[h][:], bk_t[:, hsl], ones_row[:],
                                             start=False, stop=True)
                            nc.scalar.copy(qT[h][:, s0:s0 + SQW], psq[h][:])
                            nc.scalar.copy(kT[h][:, s0:s0 + SQW], psk[h][:])
                        for s2 in range(BLK):
                            half = slice((s2 % 2) * DSH, (s2 % 2 + 1) * DSH)
                            nc.tensor.matmul(psv[s2 // 2][:, half],
                                             ones_row[:, :P], bv_t[:, :DSH],
                                             start=False, stop=(s2 % 2 == 1))
                        for s2 in range(BLK):
                            half = slice((s2 % 2) * DSH, (s2 % 2 + 1) * DSH)
                            sb = st * BLK + s2
                            vstage = stagep.tile([P, DSH], F32,
                                                 name="vstage", tag="vstage")
                            nc.vector.tensor_copy(vstage[:], psv[s2 // 2][:, half])
                            nc.sync.dma_start(v_out[b, sb * P:(sb + 1) * P, :],
                                              vstage[:])
                            for h in range(HPC):
                                nc.vector.tensor_copy(
                                    vbf[h][:, sb * HD:(sb + 1) * HD],
                                    psv[s2 // 2][:, (s2 % 2) * DSH + h * HD:
                                                 (s2 % 2) * DSH + (h + 1) * HD])
                    # k natural layout for the k output
                    for sb in range(NSB):
                        kstage = stagep.tile([P, DSH], F32,
                                             name="kstage", tag="kstage")
                        for h in range(HPC):
                            pt = ps1t.tile([P, P], F32, name="pst", tag="pst")
                            nc.tensor.transpose(
                                pt[:], kT[h][:, sb * P:(sb + 1) * P].bitcast(F32),
                                identity[:])
                            nc.scalar.copy(kstage[:, h * HD:(h + 1) * HD], pt[:])
                        nc.sync.dma_start(k_out[b, sb * P:(sb + 1) * P, :],
                                          kstage[:])

                # ---- phase 2 fused with out_proj ----
                if b == 0:
                    for h in range(HPC):
                        nc.sync.dma_start(wo_t[h][:], woT[h * P:(h + 1) * P, :])
                with tc.tile_pool(name="ps2", bufs=2, space="PSUM") as ps2, \
                     tc.tile_pool(name="ps2c", bufs=1, space="PSUM") as ps2c, \
                     tc.tile_pool(name="ps3", bufs=2, space="PSUM") as ps3:
                    for st in range(NSTW):
                        s0 = st * SQW
                        n_chunks = min(NCH, (st + 1) * BLK) if causal else NCH
                        psC = [ps2c.tile([P, SQW], F32,
                                         name=f"psC{h}", tag=f"psC{h}")
                               for h in range(HPC)]
                        psDr = [ps2c.tile([1, SQW], F32,
                                          name=f"psDr{h}", tag=f"psDr{h}")
                                for h in range(HPC)]
                        if causal:
                            mdg_s = maskp.tile([P, BLK * SQW], F32,
                                               name="mdg_s", tag="mdg_s")
                            nc.sync.dma_start(
                                mdg_s[:], mdg[:, st * BLK * SQW:
                                              (st + 1) * BLK * SQW])
                        for c in range(n_chunks):
                            diag = c * P + P - 1 >= s0
                            msrc = None
                            if causal:
                                if diag:
                                    i = c - st * BLK
                                    msrc = mdg_s[:, i * SQW:(i + 1) * SQW]
                            else:
                                mt = maskp.tile([P, SQW], F32,
                                                name="mask", tag="mask")
                                nc.sync.dma_start(
                                    mt[:], mT[c * P:(c + 1) * P, s0:s0 + SQW])
                                msrc = mt[:]
                            for h in range(HPC):
                                ss = ps2.tile([P, SQW], F32,
                                              name="psS", tag="psS")
                                nc.tensor.matmul(
                                    ss[:], kT[h][:, c * P:(c + 1) * P],
                                    qT[h][:, s0:s0 + SQW],
                                    start=True, stop=True)
                                if msrc is not None:
                                    nc.vector.tensor_add(ss[:], ss[:], msrc)
                                eg = expsp.tile([P, SQW], PV_DT,
                                                name=f"eg{h}", tag=f"eg{h}",
                                                bufs=3)
                                nc.scalar.activation(eg[:], ss[:], AF.Exp)
                                nc.tensor.matmul(
                                    psC[h][:],
                                    vbf[h][:, c * HD:(c + 1) * HD], eg[:],
                                    start=(c == 0), stop=(c == n_chunks - 1))
                                nc.tensor.matmul(
                                    psDr[h][:], ones_col[:], eg[:],
                                    start=(c == 0),
                                    stop=(c == n_chunks - 1))
                        # normalize: ctx^T[d, s_q] *= (1/denom)[s_q]
                        for h in range(HPC):
                            rrow = stagep.tile([1, SQW], F32R,
                                               name="rrow", tag="rrow")
                            with nc.allow_low_precision(
                                    reason="1/denom rounded to f32r for the "
                                           "K=1 broadcast matmul"):
                                nc.vector.reciprocal(rrow[:], psDr[h][:])
                            bc = ps2.tile([P, SQW], F32, name="psS", tag="psS")
                            nc.tensor.matmul(bc[:], ones_row[:1, :P], rrow[:],
                                             start=True, stop=True)
                            bcs = stagep.tile([P, SQW], F32,
                                              name="bcs", tag="bcs")
                            nc.vector.tensor_copy(bcs[:], bc[:])
                            nc.vector.tensor_mul(
                                ctxT[h][:, s0:s0 + SQW], psC[h][:], bcs[:])
                        for blk in range(BLK):
                            sb = st * BLK + blk
                            for mst in range(NSTW):
                                po = ps3.tile([P, SQW], F32, name="psO", tag="psO")
                                for h in range(HPC):
                                    nc.tensor.matmul(
                                        po[:], ctxT[h][:, sb * P:(sb + 1) * P],
                                        wo_t[h][:, mst * SQW:(mst + 1) * SQW],
                                        start=(h == 0), stop=(h == HPC - 1))
                                ostage = stagep.tile([P, SQW], F32,
                                                     name="ostage", tag="ostage")
                                if (sb + mst) % 2 == 0:
                                    nc.vector.tensor_copy(ostage[:], po[:])
                                else:
                                    nc.scalar.copy(ostage[:], po[:])
                                nc.sync.dma_start(
                                    outp[b * S + sb * P: b * S + (sb + 1) * P,
                                         mst * SQW:(mst + 1) * SQW], ostage[:])

    if split:
        _split_multiwait_insts(nc)
    return nc


_PROGRAMS = {}


def _get_program(causal):
    if causal not in _PROGRAMS:
        _PROGRAMS[causal] = build_program(causal=causal)
    return _PROGRAMS[causal]


def _is_causal_mask(mask2d):
    """True iff mask2d is an additive causal mask: zeros on and below the
    diagonal, <= -1e9 strictly above."""
    iu = np.triu_indices(S, 1)
    if not (mask2d[iu] <= -1e9 + 1).all():
        return False
    il = np.tril_indices(S, 0)
    return bool((mask2d[il] == 0.0).all())


def _register_ntff_hook():
    """The image's antenv lacks axon_hooks; give bass_utils a working one."""
    try:
        import antenv.axon_hooks  # noqa: F401
        return
    except ImportError:
        pass
    try:
        import trn_agent_boot.trn_boot as tb
        hook = tb._ntff_profile_via_ctypes("/opt/axon/libaxon_pjrt.so")
    except Exception:
        hook = None
    mod = types.ModuleType("antenv.axon_hooks")
    mod.get_axon_ntff_profile_hook = lambda: hook
    mod.set_axon_ntff_profile_hook = lambda h: None
    sys.modules["antenv.axon_hooks"] = mod


def make_in_maps(hs, mask, Wq, bq, Wk, bk, Wv, bv, Wo, bo):
    f32 = np.float32
    sc = f32(1.0 / np.sqrt(HD))
    x = hs.reshape(B * S, D)
    causal = _is_causal_mask(mask[0, 0])
    mTc = np.maximum(mask[0, 0].T, MASK_CLAMP).astype(f32)
    # hidden^T packed per (b, strip): chunk-major [128, 512] blocks
    hTn = x.T.reshape(NCH, P, B, NSTW, SQW)          # [c, p, b, st, w]
    hPack = np.ascontiguousarray(
        hTn.transpose(1, 2, 3, 0, 4).reshape(P, B * NSTW * NCH * SQW))
    # diagonal mask blocks [128, NSTW*BLK*SQW]
    mdg = np.zeros((P, NSTW * BLK * SQW), f32)
    for st in range(NSTW):
        for i in range(BLK):
            c = st * BLK + i
            mdg[:, (st * BLK + i) * SQW:(st * BLK + i + 1) * SQW] = \
                mTc[c * P:(c + 1) * P, st * SQW:(st + 1) * SQW]
    ident = np.eye(P, dtype=f32)

    def wpack(Wsh):  # [DSH, D] row-shard -> packed [128, NCH*DSH]
        t = np.ascontiguousarray(Wsh.T)  # [D, DSH]
        return np.ascontiguousarray(
            t.reshape(NCH, P, DSH).transpose(1, 0, 2).reshape(P, NCH * DSH))

    def brow(xx):
        row = np.zeros(SQW, f32)
        row[:DSH] = xx
        return row

    in_maps = []
    for c in range(NCORES):
        r = slice(c * DSH, (c + 1) * DSH)
        in_maps.append({
            "hT": hPack,
            "mT": np.ascontiguousarray(mTc),
            "mdg": mdg,
            "wqT": wpack((Wq[r, :] * sc).astype(f32)),
            "wkT": wpack(Wk[r, :]),
            "wvT": wpack(Wv[r, :]),
            "woT": np.ascontiguousarray(Wo[:, r].T),
            "bqv": np.ascontiguousarray(
                np.stack([brow(bq[r] * sc), brow(bk[r]), brow(bv[r]),
                          np.ones(SQW, f32)])),
            "ident_d": ident,
        })
    return in_maps, causal


def run_sharded(hidden_states, attn_mask, Wq, bq, Wk, bk, Wv, bv, Wo, bo,
                trace=False):
    """Shard inputs, run the 8-core SPMD kernel, gather. Returns
    ((out, k, v), BassKernelResults)."""
    _register_ntff_hook()
    f32 = np.float32
    hs = np.asarray(hidden_states, f32)
    mask = np.asarray(attn_mask, f32)
    Wq, bq = np.asarray(Wq, f32), np.asarray(bq, f32)
    Wk, bk = np.asarray(Wk, f32), np.asarray(bk, f32)
    Wv, bv = np.asarray(Wv, f32), np.asarray(bv, f32)
    Wo, bo = np.asarray(Wo, f32), np.asarray(bo, f32)

    in_maps, causal = make_in_maps(hs, mask, Wq, bq, Wk, bk, Wv, bv, Wo, bo)
    nc = _get_program(causal)
    res = run_bass_kernel_spmd(nc, in_maps, core_ids=list(range(NCORES)),
                               trace=trace)

    out = np.zeros((B * S, D), f32)
    k = np.empty((B, H, S, HD), f32)
    v = np.empty((B, H, S, HD), f32)
    for c in range(NCORES):
        out += res.results[c]["outp"]
        ksh = res.results[c]["k_out"].reshape(B, S, HPC, HD)
        vsh = res.results[c]["v_out"].reshape(B, S, HPC, HD)
        k[:, c * HPC:(c + 1) * HPC] = ksh.transpose(0, 2, 1, 3)
        v[:, c * HPC:(c + 1) * HPC] = vsh.transpose(0, 2, 1, 3)
    out = (out + bo).reshape(B, S, D).astype(f32)
    return (out, k, v), res


def kernel(hidden_states, attn_mask, Wq, bq, Wk, bk, Wv, bv, Wo, bo):
    (out, k, v), _ = run_sharded(hidden_states, attn_mask,
                                 Wq, bq, Wk, bk, Wv, bv, Wo, bo)
    return out, k, v


# revision 18
# speedup vs baseline: 1.0577x; 1.0577x over previous
"""Multi-head attention (B=2, S=2048, D=2048, H=16) on 8 TRN2 NeuronCores.

Tensor-parallel over heads: each core computes 2 of 16 heads end to end
(q/k/v projections column-sharded, out_proj row-sharded) and writes a
partial output; the host sums the 8 partials and adds the output bias.

Per-core device pipeline (b = batch index, looped):
  phase 1: q^T,k^T ([HD, S] layout) and v ([S, HD] layout) via matmuls
           against host-pre-transposed hidden^T; biases are added with a
           K=1 matmul against a ones row. k is PE-transposed to [S, HD]
           for the k output.
  phase 2 (fused with out_proj, per 512-wide s_q strip):
           scores^T[s_k, s_q] = k^T.T @ q^T (1/sqrt(HD) folded into Wq),
           + mask^T on diagonal blocks (DVE), exp (ACT, bf16 out), then
           ctx^T[d, s_q] += V_chunk.T @ exp(S^T) and a denominator row
           denom[1, s_q] += ones.T @ exp(S^T), both accumulated in PSUM
           chunk by chunk. 1/denom is broadcast across partitions with a
           K=1 matmul and ctx^T is normalized by one DVE multiply; the
           out_proj partial for the strip follows immediately.

Matmuls for projections/scores/out_proj run in float32r (single-pass
fp32 on the PE at bf16 rate); the PV matmul runs in bf16 on softmax
weights in [0,1]. All accumulation is fp32 in PSUM. A causal-specialized
program skips fully-masked score blocks; any other mask takes a general
program with identical math.
"""

import os
import sys
import types

for _p in ("/opt/trn_rl_repo", "/root/.axon_site/_ro/trn_rl_repo", "/root/.axon_site"):
    if os.path.isdir(_p) and _p not in sys.path:
        sys.path.append(_p)

import numpy as np

import concourse.bass as bass
import concourse.mybir as mybir
import concourse.tile as tile
from concourse.bass_utils import run_bass_kernel_spmd

# problem shape (hardcoded per contest contract)
B, S, D, H, HD = 2, 2048, 2048, 16, 128
NCORES = 8
HPC = H // NCORES          # heads per core = 2
DSH = HPC * HD             # per-core model-dim shard = 256
P = 128                    # partitions
NCH = D // P               # contraction chunks = 16
NSB = S // P               # 128-row blocks per batch = 16
SQW = 512                  # matmul free width
NSTW = S // SQW            # strips per batch = 4
BLK = SQW // P             # 128-blocks per strip = 4
CG = 4                     # hidden chunks per packed DMA

F32 = mybir.dt.float32
F32R = mybir.dt.float32r   # projection/scores/out_proj matmul dtype
PV_DT = mybir.dt.bfloat16  # exp(S) and V dtypes for the PV matmul
AF = mybir.ActivationFunctionType

MASK_CLAMP = -60.0


def _split_multiwait_insts(nc):
    """This walrus build rejects any instruction carrying more than one
    sync-wait command (seen on Drain/CTRL and Matmult/S3_LW). Tile
    occasionally aggregates several waits onto one instruction. Hoist all
    but the last wait onto preceding NoOps on the same engine."""
    n = [0]

    def fresh(base):
        n[0] += 1
        return f"{base}_wsplit{n[0]}"

    for fn in nc.m.functions:
        for blk in fn.blocks:
            out = []
            for inst in blk.instructions:
                si = getattr(inst, "sync_info", None)
                if si is not None and len(si.on_wait) > 1:
                    waits = list(si.on_wait)
                    for w in waits[:-1]:
                        d = mybir.InstNoOp(name=fresh(inst.name), ins=[], outs=[])
                        d.engine = inst.engine
                        d.sync_info = mybir.SyncInfo(on_wait=[w], on_update=[])
                        out.append(d)
                    inst.sync_info = mybir.SyncInfo(
                        on_wait=[waits[-1]], on_update=list(si.on_update)
                    )
                out.append(inst)
            blk.instructions[:] = out


def build_program(causal=True, split=True):
    nc = bass.Bass(target_bir_lowering=False, num_swdge_queues=4)

    hT = nc.dram_tensor("hT", [P, B * NSTW * NCH * SQW], F32R,
                        kind="ExternalInput")
    mT = nc.dram_tensor("mT", [S, S], F32, kind="ExternalInput")
    # diagonal mask blocks, packed [128, NSTW*BLK*SQW] (causal path)
    mdg = nc.dram_tensor("mdg", [P, NSTW * BLK * SQW], F32, kind="ExternalInput")
    # projection weights, packed [128, NCH*DSH] chunk-major
    wqT = nc.dram_tensor("wqT", [P, NCH * DSH], F32R, kind="ExternalInput")
    wkT = nc.dram_tensor("wkT", [P, NCH * DSH], F32R, kind="ExternalInput")
    wvT = nc.dram_tensor("wvT", [P, NCH * DSH], F32R, kind="ExternalInput")
    woT = nc.dram_tensor("woT", [DSH, D], F32R, kind="ExternalInput")
    # rows: bq (pre-scaled), bk, bv, ones
    bqv = nc.dram_tensor("bqv", [4, SQW], F32R, kind="ExternalInput")
    ident_d = nc.dram_tensor("ident_d", [P, P], F32, kind="ExternalInput")
    outp = nc.dram_tensor("outp", [B * S, D], F32, kind="ExternalOutput")
    k_out = nc.dram_tensor("k_out", [B, S, DSH], F32, kind="ExternalOutput")
    v_out = nc.dram_tensor("v_out", [B, S, DSH], F32, kind="ExternalOutput")

    with tile.TileContext(nc) as tc:
        from contextlib import ExitStack

        with ExitStack() as top:
            constp = top.enter_context(tc.tile_pool(name="const", bufs=1))
            wproj = top.enter_context(tc.tile_pool(name="wproj", bufs=1))
            qkvp = top.enter_context(tc.tile_pool(name="qkv", bufs=1))
            streamp = top.enter_context(tc.tile_pool(name="stream", bufs=2))
            expsp = top.enter_context(tc.tile_pool(name="exps", bufs=1))
            maskp = top.enter_context(tc.tile_pool(name="mask", bufs=3))
            stagep = top.enter_context(tc.tile_pool(name="stage", bufs=3))

            identity = constp.tile([P, P], F32, name="identity", tag="identity")
            nc.sync.dma_start(identity[:], ident_d[:, :])

            wq_t = wproj.tile([P, NCH * DSH], F32R, name="wq_t", tag="wq_t")
            wk_t = wproj.tile([P, NCH * DSH], F32R, name="wk_t", tag="wk_t")
            wv_t = wproj.tile([P, NCH * DSH], F32R, name="wv_t", tag="wv_t")
            for t, dram in ((wq_t, wqT), (wk_t, wkT), (wv_t, wvT)):
                nc.sync.dma_start(t[:], dram[:, :])

            bq_t = constp.tile([1, SQW], F32R, name="bq_t", tag="bq_t")
            bk_t = constp.tile([1, SQW], F32R, name="bk_t", tag="bk_t")
            bv_t = constp.tile([1, SQW], F32R, name="bv_t", tag="bv_t")
            ones_row = constp.tile([1, SQW], F32R, name="ones_row", tag="ones_row")
            for i, t in enumerate((bq_t, bk_t, bv_t, ones_row)):
                nc.sync.dma_start(t[:], bqv[i:i + 1, :])
            ones_col = constp.tile([P, 1], PV_DT, name="ones_col", tag="ones_col")
            nc.vector.memset(ones_col[:], 1.0)

            # out_proj weights and the resident diagonal mask load later --
            # first needed well into batch 0
            wo_t = [constp.tile([P, D], F32R, name=f"woT{h}", tag=f"woT{h}")
                    for h in range(HPC)]


            def wslice(t, c):
                return t[:, c * DSH:(c + 1) * DSH]

            for b in range(B):
                qT = [qkvp.tile([P, S], F32R, name=f"qT{h}", tag=f"qT{h}")
                      for h in range(HPC)]
                kT = [qkvp.tile([P, S], F32R, name=f"kT{h}", tag=f"kT{h}")
                      for h in range(HPC)]
                vbf = [qkvp.tile([P, NCH * HD], PV_DT,
                                 name=f"vbf{h}", tag=f"vbf{h}")
                       for h in range(HPC)]
                ctxT = [qkvp.tile([P, S], F32R, name=f"ctxT{h}", tag=f"ctxT{h}")
                        for h in range(HPC)]

                # ---- phase 1: projections ----
                with tc.tile_pool(name="ps1", bufs=1, space="PSUM") as ps1, \
                     tc.tile_pool(name="ps1t", bufs=2, space="PSUM") as ps1t:
                    for st in range(NSTW):
                        s0 = st * SQW
                        psq = [ps1.tile([P, SQW], F32, name=f"psq{h}", tag=f"psq{h}")
                               for h in range(HPC)]
                        psk = [ps1.tile([P, SQW], F32, name=f"psk{h}", tag=f"psk{h}")
                               for h in range(HPC)]
                        psv = [ps1.tile([P, SQW], F32, name=f"psv{i}", tag=f"psv{i}")
                               for i in range(2)]
                        base = (b * NSTW + st) * NCH * SQW
                        for cg in range(NCH // CG):
                            hc = streamp.tile([P, CG * SQW], F32R,
                                              name="hchunk", tag="hchunk")
                            nc.sync.dma_start(
                                hc[:], hT[:, base + cg * CG * SQW:
                                          base + (cg + 1) * CG * SQW])
                            for ci in range(CG):
                                c = cg * CG + ci
                                hcs = hc[:, ci * SQW:(ci + 1) * SQW]
                                for h in range(HPC):
                                    hsl = slice(h * HD, (h + 1) * HD)
                                    nc.tensor.matmul(
                                        psq[h][:], wslice(wq_t, c)[:, hsl], hcs,
                                        start=(c == 0), stop=False)
                                    nc.tensor.matmul(
                                        psk[h][:], wslice(wk_t, c)[:, hsl], hcs,
                                        start=(c == 0), stop=False)
                                for s2 in range(BLK):
                                    nc.tensor.matmul(
                                        psv[s2 // 2][:, (s2 % 2) * DSH:
                                                     (s2 % 2 + 1) * DSH],
                                        hcs[:, s2 * P:(s2 + 1) * P],
                                        wslice(wv_t, c),
                                        start=(c == 0 and s2 % 2 == 0),
                                        stop=False)
                        for h in range(HPC):
                            hsl = slice(h * HD, (h + 1) * HD)
                            nc.tensor.matmul(psq[h][:], bq_t[:, hsl], ones_row[:],
                                             start=False, stop=True)
                            nc.tensor.matmul(psk[h][:], bk_t[:, hsl], ones_row[:],
                                             start=False, stop=True)
                            nc.scalar.copy(qT[h][:, s0:s0 + SQW], psq[h][:])
                            nc.scalar.copy(kT[h][:, s0:s0 + SQW], psk[h][:])
                        for s2 in range(BLK):
                            half = slice((s2 % 2) * DSH, (s2 % 2 + 1) * DSH)
                            nc.tensor.matmul(psv[s2 // 2][:, half],
                                             ones_row[:, :P], bv_t[:, :DSH],
                                             start=False, stop=(s2 % 2 == 1))
                        for s2 in range(BLK):
                            half = slice((s2 % 2) * DSH, (s2 % 2 + 1) * DSH)
                            sb = st * BLK + s2
                            vstage = stagep.tile([P, DSH], F32,
                                                 name="vstage", tag="vstage")
                            nc.vector.tensor_copy(vstage[:], psv[s2 // 2][:, half])
                            nc.sync.dma_start(v_out[b, sb * P:(sb + 1) * P, :],
                                              vstage[:])
                            for h in range(HPC):
                                nc.vector.tensor_copy(
                                    vbf[h][:, sb * HD:(sb + 1) * HD],
                                    psv[s2 // 2][:, (s2 % 2) * DSH + h * HD:
                                                 (s2 % 2) * DSH + (h + 1) * HD])
                    # k natural layout for the k output
                    for sb in range(NSB):
                        kstage = stagep.tile([P, DSH], F32,
                                             name="kstage", tag="kstage")
                        for h in range(HPC):
                            pt = ps1t.tile([P, P], F32, name="pst", tag="pst")
                            nc.tensor.transpose(
                                pt[:], kT[h][:, sb * P:(sb + 1) * P].bitcast(F32),
                                identity[:])
                            nc.scalar.copy(kstage[:, h * HD:(h + 1) * HD], pt[:])
                        nc.sync.dma_start(k_out[b, sb * P:(sb + 1) * P, :],
                                          kstage[:])

                # ---- phase 2 fused with out_proj ----
                if b == 0:
                    for h in range(HPC):
                        nc.sync.dma_start(wo_t[h][:], woT[h * P:(h + 1) * P, :])
                with tc.tile_pool(name="ps2", bufs=2, space="PSUM") as ps2, \
                     tc.tile_pool(name="ps2c", bufs=1, space="PSUM") as ps2c, \
                     tc.tile_pool(name="ps3", bufs=2, space="PSUM") as ps3:
                    for st in range(NSTW):
                        s0 = st * SQW
                        n_chunks = min(NCH, (st + 1) * BLK) if causal else NCH
                        psC = [ps2c.tile([P, SQW], F32,
                                         name=f"psC{h}", tag=f"psC{h}")
                               for h in range(HPC)]
                        psDr = [ps2c.tile([1, SQW], F32,
                                          name=f"psDr{h}", tag=f"psDr{h}")
                                for h in range(HPC)]
                        if causal:
                            mdg_s = maskp.tile([P, BLK * SQW], F32,
                                               name="mdg_s", tag="mdg_s")
                            nc.sync.dma_start(
                                mdg_s[:], mdg[:, st * BLK * SQW:
                                              (st + 1) * BLK * SQW])
                        # software pipeline: PV/denominator trail the
                        # scores->exp chain by PV_DELAY chunks so the PE
                        # always has score matmuls between PV waits
                        PV_DELAY = 2
                        pending = []

                        def emit_pv(c, egs_c):
                            for h in range(HPC):
                                nc.tensor.matmul(
                                    psC[h][:],
                                    vbf[h][:, c * HD:(c + 1) * HD], egs_c[h],
                                    start=(c == 0), stop=(c == n_chunks - 1))
                                nc.tensor.matmul(
                                    psDr[h][:], ones_col[:], egs_c[h],
                                    start=(c == 0),
                                    stop=(c == n_chunks - 1))

                        for c in range(n_chunks):
                            diag = c * P + P - 1 >= s0
                            msrc = None
                            if causal:
                                if diag:
                                    i = c - st * BLK
                                    msrc = mdg_s[:, i * SQW:(i + 1) * SQW]
                            else:
                                mt = maskp.tile([P, SQW], F32,
                                                name="mask", tag="mask")
                                nc.sync.dma_start(
                                    mt[:], mT[c * P:(c + 1) * P, s0:s0 + SQW])
                                msrc = mt[:]
                            egs_c = []
                            for h in range(HPC):
                                ss = ps2.tile([P, SQW], F32,
                                              name="psS", tag="psS")
                                nc.tensor.matmul(
                                    ss[:], kT[h][:, c * P:(c + 1) * P],
                                    qT[h][:, s0:s0 + SQW],
                                    start=True, stop=True)
                                if msrc is not None:
                                    nc.vector.tensor_add(ss[:], ss[:], msrc)
                                eg = expsp.tile([P, SQW], PV_DT,
                                                name=f"eg{h}", tag=f"eg{h}",
                                                bufs=PV_DELAY + 2)
                                nc.scalar.activation(eg[:], ss[:], AF.Exp)
                                egs_c.append(eg[:])
                            pending.append((c, egs_c))
                            if len(pending) > PV_DELAY:
                                emit_pv(*pending.pop(0))
                        for item in pending:
                            emit_pv(*item)
                        # normalize: ctx^T[d, s_q] *= (1/denom)[s_q]
                        for h in range(HPC):
                            rrow = stagep.tile([1, SQW], F32R,
                                               name="rrow", tag="rrow")
                            with nc.allow_low_precision(
                                    reason="1/denom rounded to f32r for the "
                                           "K=1 broadcast matmul"):
                                nc.vector.reciprocal(rrow[:], psDr[h][:])
                            bc = ps2.tile([P, SQW], F32, name="psS", tag="psS")
                            nc.tensor.matmul(bc[:], ones_row[:1, :P], rrow[:],
                                             start=True, stop=True)
                            bcs = stagep.tile([P, SQW], F32,
                                              name="bcs", tag="bcs")
                            nc.vector.tensor_copy(bcs[:], bc[:])
                            nc.vector.tensor_mul(
                                ctxT[h][:, s0:s0 + SQW], psC[h][:], bcs[:])
                        for blk in range(BLK):
                            sb = st * BLK + blk
                            for mst in range(NSTW):
                                po = ps3.tile([P, SQW], F32, name="psO", tag="psO")
                                for h in range(HPC):
                                    nc.tensor.matmul(
                                        po[:], ctxT[h][:, sb * P:(sb + 1) * P],
                                        wo_t[h][:, mst * SQW:(mst + 1) * SQW],
                                        start=(h == 0), stop=(h == HPC - 1))
                                ostage = stagep.tile([P, SQW], F32,
                                                     name="ostage", tag="ostage")
                                if (sb + mst) % 2 == 0:
                                    nc.vector.tensor_copy(ostage[:], po[:])
                                else:
                                    nc.scalar.copy(ostage[:], po[:])
                                nc.sync.dma_start(
                                    outp[b * S + sb * P: b * S + (sb + 1) * P,
                                         mst * SQW:(mst + 1) * SQW], ostage[:])

    if split:
        _split_multiwait_insts(nc)
    return nc


_PROGRAMS = {}


def _get_program(causal):
    if causal not in _PROGRAMS:
        _PROGRAMS[causal] = build_program(causal=causal)
    return _PROGRAMS[causal]


def _is_causal_mask(mask2d):
    """True iff mask2d is an additive causal mask: zeros on and below the
    diagonal, <= -1e9 strictly above."""
    iu = np.triu_indices(S, 1)
    if not (mask2d[iu] <= -1e9 + 1).all():
        return False
    il = np.tril_indices(S, 0)
    return bool((mask2d[il] == 0.0).all())


def _register_ntff_hook():
    """The image's antenv lacks axon_hooks; give bass_utils a working one."""
    try:
        import antenv.axon_hooks  # noqa: F401
        return
    except ImportError:
        pass
    try:
        import trn_agent_boot.trn_boot as tb
        hook = tb._ntff_profile_via_ctypes("/opt/axon/libaxon_pjrt.so")
    except Exception:
        hook = None
    mod = types.ModuleType("antenv.axon_hooks")
    mod.get_axon_ntff_profile_hook = lambda: hook
    mod.set_axon_ntff_profile_hook = lambda h: None
    sys.modules["antenv.axon_hooks"] = mod


def make_in_maps(hs, mask, Wq, bq, Wk, bk, Wv, bv, Wo, bo):
    f32 = np.float32
    sc = f32(1.0 / np.sqrt(HD))
    x = hs.reshape(B * S, D)
    causal = _is_causal_mask(mask[0, 0])
    mTc = np.maximum(mask[0, 0].T, MASK_CLAMP).astype(f32)
    # hidden^T packed per (b, strip): chunk-major [128, 512] blocks
    hTn = x.T.reshape(NCH, P, B, NSTW, SQW)          # [c, p, b, st, w]
    hPack = np.ascontiguousarray(
        hTn.transpose(1, 2, 3, 0, 4).reshape(P, B * NSTW * NCH * SQW))
    # diagonal mask blocks [128, NSTW*BLK*SQW]
    mdg = np.zeros((P, NSTW * BLK * SQW), f32)
    for st in range(NSTW):
        for i in range(BLK):
            c = st * BLK + i
            mdg[:, (st * BLK + i) * SQW:(st * BLK + i + 1) * SQW] = \
                mTc[c * P:(c + 1) * P, st * SQW:(st + 1) * SQW]
    ident = np.eye(P, dtype=f32)

    def wpack(Wsh):  # [DSH, D] row-shard -> packed [128, NCH*DSH]
        t = np.ascontiguousarray(Wsh.T)  # [D, DSH]
        return np.ascontiguousarray(
            t.reshape(NCH, P, DSH).transpose(1, 0, 2).reshape(P, NCH * DSH))

    def brow(xx):
        row = np.zeros(SQW, f32)
        row[:DSH] = xx
        return row

    in_maps = []
    for c in range(NCORES):
        r = slice(c * DSH, (c + 1) * DSH)
        in_maps.append({
            "hT": hPack,
            "mT": np.ascontiguousarray(mTc),
            "mdg": mdg,
            "wqT": wpack((Wq[r, :] * sc).astype(f32)),
            "wkT": wpack(Wk[r, :]),
            "wvT": wpack(Wv[r, :]),
            "woT": np.ascontiguousarray(Wo[:, r].T),
            "bqv": np.ascontiguousarray(
                np.stack([brow(bq[r] * sc), brow(bk[r]), brow(bv[r]),
                          np.ones(SQW, f32)])),
            "ident_d": ident,
        })
    return in_maps, causal


def run_sharded(hidden_states, attn_mask, Wq, bq, Wk, bk, Wv, bv, Wo, bo,
                trace=False):
    """Shard inputs, run the 8-core SPMD kernel, gather. Returns
    ((out, k, v), BassKernelResults)."""
    _register_ntff_hook()
    f32 = np.float32
    hs = np.asarray(hidden_states, f32)
    mask = np.asarray(attn_mask, f32)
    Wq, bq = np.asarray(Wq, f32), np.asarray(bq, f32)
    Wk, bk = np.asarray(Wk, f32), np.asarray(bk, f32)
    Wv, bv = np.asarray(Wv, f32), np.asarray(bv, f32)
    Wo, bo = np.asarray(Wo, f32), np.asarray(bo, f32)

    in_maps, causal = make_in_maps(hs, mask, Wq, bq, Wk, bk, Wv, bv, Wo, bo)
    nc = _get_program(causal)
    res = run_bass_kernel_spmd(nc, in_maps, core_ids=list(range(NCORES)),
                               trace=trace)

    out = np.zeros((B * S, D), f32)
    k = np.empty((B, H, S, HD), f32)
    v = np.empty((B, H, S, HD), f32)
    for c in range(NCORES):
        out += res.results[c]["outp"]
        ksh = res.results[c]["k_out"].reshape(B, S, HPC, HD)
        vsh = res.results[c]["v_out"].reshape(B, S, HPC, HD)
        k[:, c * HPC:(c + 1) * HPC] = ksh.transpose(0, 2, 1, 3)
        v[:, c * HPC:(c + 1) * HPC] = vsh.transpose(0, 2, 1, 3)
    out = (out + bo).reshape(B, S, D).astype(f32)
    return (out, k, v), res


def kernel(hidden_states, attn_mask, Wq, bq, Wk, bk, Wv, bv, Wo, bo):
    (out, k, v), _ = run_sharded(hidden_states, attn_mask,
                                 Wq, bq, Wk, bk, Wv, bv, Wo, bo)
    return out, k, v
